# revision 1
# baseline (speedup 1.0000x reference)
"""MoE block (B=16, C=192, H=W=32, E=8, top-2, 3x3 same-conv experts) on 8 trn2 cores.

Strategy:
  - Router (tiny: pool -> 192x8 matmul -> softmax -> top2) computed on host in numpy.
  - Conv is linear in weights, so the top-2 expert combine folds into ONE conv
    per sample with host-combined weights:
        out[b] = conv(x[b], sum_k w_bk * W_ek) + sum_k w_bk * b_ek
    Device work: 16 convs total -> 2 per core (data-parallel over batch).
  - Each conv = 9 shifted bf16 matmuls (taps) accumulating in fp32 PSUM;
    contract =
    input channels (192 = 128 + 64), M = output channels (192 = 128 + 64),
    N = 512 pixels (half image).
  - PE-array packing: column tiling is rejected by walrus on TRN2, so only
    row tiling is used. Each K=64 leftover-channel tap runs as a row PAIR
    (tile_position rows 0/64) covering BOTH pixel blocks concurrently,
    writing two different PSUM banks. Partitions 64..127 of the TB x-tile and
    of the K64 weight tile hold duplicates of partitions 0..63.
    144 naive matmuls -> 108 PE slots.
"""

import numpy as np

B, C, H, W = 16, 192, 32, 32
E, TOPK = 8, 2
NCORES = 8
S = B // NCORES          # samples per core
PW = W + 2               # padded width 34
PP = PW * PW             # padded pixels 1156
HWP = H * W              # 1024
PBS = 512                # pixels per block
ROWS_PB = 16             # output rows per block
XROWS = [(0, 18), (15, 34)]   # padded-row range each pixel block needs
TAPS = [(t // 3, t % 3) for t in range(9)]
N_WARMUP = 12

_cache = {}


def _build_module():
    import concourse.tile as tile
    from concourse import bacc, mybir

    f32 = mybir.dt.float32
    f32r = mybir.dt.bfloat16  # compute dtype (variable name kept from the f32r variant)

    nc = bacc.Bacc("TRN2", target_bir_lowering=False, debug=False, num_devices=NCORES)
    xp_d = nc.dram_tensor("xp", [S, C, PP], f32r, kind="ExternalInput")
    wa_d = nc.dram_tensor("wa", [S, 128, 9 * C], f32r, kind="ExternalInput")
    wbb_d = nc.dram_tensor("wbb", [S, 64, 9 * C], f32r, kind="ExternalInput")
    bias_d = nc.dram_tensor("bias", [128, 4], f32, kind="ExternalInput")
    out_d = nc.dram_tensor("out", [S, C, HWP], f32, kind="ExternalOutput")

    with tile.TileContext(nc) as tc:
        with (
            tc.tile_pool(name="xin", bufs=1) as xin,
            tc.tile_pool(name="win", bufs=1) as win,
            tc.tile_pool(name="cst", bufs=1) as cst,
            tc.tile_pool(name="ps", bufs=3, space="PSUM") as ps,
            tc.tile_pool(name="pw", bufs=1, space="PSUM") as pw,
            tc.tile_pool(name="oev", bufs=4) as oev,
        ):
            # --- PE warmup: tiny matmuls on zeros keep the clock ramped while
            # input DMAs stream in.
            scr = cst.tile([128, 512], mybir.dt.bfloat16, name="scr", tag="scr")
            nc.vector.memset(scr[:], 0.0)
            ps_scr = pw.tile([128, 512], f32, name="ps_scr", tag="ps_scr")
            for i in range(N_WARMUP):
                nc.tensor.matmul(ps_scr[:], scr[:, 0:128], scr[:], start=True,
                                 stop=True, skip_group_check=True)

            bias_t = cst.tile([128, 4], f32, name="bias_t", tag="bias_t")

            Ta = {}   # (s, pb) -> [128, rows*34] ch0-127 chunk
            TB = {}   # s -> [128, 1156]: ch128-191, duplicated on both halves
            WaC = {}  # (s, c) -> weight chunks for taps 0-2 / 3-8
            WBB = {}  # s -> [128, 9*192] K64 weights, duplicated halves

            def emit_input_dmas(s):
                ta0 = xin.tile([128, 18 * PW], f32r, name=f"Ta{s}_0", tag=f"Ta{s}_0")
                nc.sync.dma_start(ta0[:], xp_d[s, 0:128, 0 : 18 * PW])
                Ta[(s, 0)] = ta0
                # A-block weights in three tap-chunks, interleaved across the
                # ACT and SP issue paths so arrival order matches tap order.
                for ch, eng in ((0, nc.scalar), (1, nc.sync), (2, nc.scalar)):
                    wac = win.tile([128, 3 * C], f32r, name=f"WaC{s}_{ch}",
                                   tag=f"WaC{s}_{ch}")
                    eng.dma_start(wac[:], wa_d[s, :, ch * 3 * C : (ch + 1) * 3 * C])
                    WaC[(s, ch)] = wac

                ta1 = xin.tile([128, 19 * PW], f32r, name=f"Ta{s}_1", tag=f"Ta{s}_1")
                nc.sync.dma_start(ta1[:], xp_d[s, 0:128, 15 * PW : 34 * PW])
                Ta[(s, 1)] = ta1

                # Lower half serves pixel-block-0 windows (padded rows 0..17),
                # upper half serves pixel-block-1 windows (rows 15..33) -- so
                # each half only needs its row range; no duplicate bytes.
                tb = xin.tile([128, PP], f32r, name=f"TB_{s}", tag=f"TB_{s}")
                nc.sync.dma_start(tb[0:64, 0 : 18 * PW], xp_d[s, 128:192, 0 : 18 * PW])
                nc.gpsimd.dma_start(tb[64:128, 15 * PW : PP], xp_d[s, 128:192, 15 * PW : PP])
                TB[s] = tb

                wbb = win.tile([128, 9 * C], f32r, name=f"WBB{s}", tag=f"WBB{s}")
                nc.gpsimd.dma_start(wbb[0:64, :], wbb_d[s])
                nc.gpsimd.dma_start(wbb[64:128, :], wbb[0:64, :])
                WBB[s] = wbb
                if s == 0:
                    nc.sync.dma_start(bias_t[:], bias_d[:])

            def wa_tap(s, t):
                """lhsT slice for tap t: [K=128, M=192] within its chunk."""
                return WaC[(s, t // 3)][:, (t % 3) * C : (t % 3 + 1) * C]

            def ta_rhs(s, pb, t):
                dy, dx = TAPS[t]
                v = Ta[(s, pb)][:].rearrange("p (r c) -> p r c", c=PW)
                y = ROWS_PB * pb - XROWS[pb][0] + dy
                return v[:, y : y + ROWS_PB, dx : dx + W]

            def tb_rhs(s, half, pb, t):
                """K64 moving AP from the duplicated TB tile: partition half
                `half` (0 -> rows 0..63, 1 -> 64..127), tap-t window of pb."""
                dy, dx = TAPS[t]
                v = TB[s][:].rearrange("p (r c) -> p r c", c=PW)
                y = ROWS_PB * pb + dy
                return v[64 * half : 64 * half + 64, y : y + ROWS_PB, dx : dx + W]

            def emit_A(s, pb, psA_pb):
                for t in range(9):
                    nc.tensor.matmul(psA_pb[:], wa_tap(s, t)[:, 0:128],
                                     ta_rhs(s, pb, t), start=(t == 0), stop=False)

            def emit_B(s, psA):
                # tap t covers pixel blocks 0 (array rows 0..63) and 1
                # (rows 64..127) concurrently, into two different PSUM banks.
                for t in range(9):
                    nc.tensor.matmul(psA[0][:], WBB[s][0:64, t * C : t * C + 128],
                                     tb_rhs(s, 0, 0, t),
                                     start=False, stop=(t == 8))
                    nc.tensor.matmul(psA[1][:], WBB[s][64:128, t * C : t * C + 128],
                                     tb_rhs(s, 1, 1, t),
                                     start=False, stop=(t == 8))

            def emit_C(s, pb, psC_pb):
                for t in range(9):
                    nc.tensor.matmul(psC_pb[:], wa_tap(s, t)[:, 128:192],
                                     ta_rhs(s, pb, t), start=(t == 0), stop=False)

            def emit_D(s, psC):
                # row-paired like B: pb0 on rows 0..63, pb1 on rows 64..127.
                for t in range(9):
                    nc.tensor.matmul(psC[0][:], WBB[s][0:64, t * C + 128 : t * C + 192],
                                     tb_rhs(s, 0, 0, t),
                                     start=False, stop=(t == 8))
                    nc.tensor.matmul(psC[1][:], WBB[s][64:128, t * C + 128 : t * C + 192],
                                     tb_rhs(s, 1, 1, t),
                                     start=False, stop=(t == 8))

            def evict_A(s, pb, psA_pb):
                # DVE: PSUM + per-channel bias -> SBUF, then DMA out.
                oA = oev.tile([128, PBS], f32, name=f"oA_{s}_{pb}", tag="oA")
                nc.vector.tensor_scalar_add(oA[:], psA_pb[:], bias_t[:, s : s + 1])
                eng = nc.scalar if s == 0 else nc.sync
                eng.dma_start(out_d[s, 0:128, pb * PBS : (pb + 1) * PBS], oA[:])

            def evict_C(s, psC):
                # ACT: two [64,512] banks -> one [64,1024] tile, one DMA out.
                oC = oev.tile([64, HWP], f32, name=f"oC_{s}", tag="oC")
                for pb in range(2):
                    nc.scalar.activation(
                        oC[:, pb * PBS : (pb + 1) * PBS], psC[pb][:],
                        mybir.ActivationFunctionType.Identity,
                        bias=bias_t[0:64, 2 + s : 3 + s], scale=1.0,
                    )
                eng = nc.gpsimd if s == 0 else nc.sync
                eng.dma_start(out_d[s, 128:192, :], oC[:])

            def emit_sample(s):
                psA = [
                    ps.tile([128, PBS], f32, name=f"psA_{s}_{pb}", tag="psA")
                    for pb in range(2)
                ]
                psC = [
                    ps.tile([64, PBS], f32, name=f"psC_{s}_{pb}", tag="psC")
                    for pb in range(2)
                ]
                emit_A(s, 0, psA[0])
                emit_A(s, 1, psA[1])
                emit_C(s, 0, psC[0])
                emit_C(s, 1, psC[1])
                emit_B(s, psA)
                evict_A(s, 0, psA[0])
                evict_A(s, 1, psA[1])
                emit_D(s, psC)
                evict_C(s, psC)

            emit_input_dmas(0)
            emit_input_dmas(1)
            emit_sample(0)
            emit_sample(1)

    nc.compile()
    return nc


def get_module():
    if "nc" not in _cache:
        _cache["nc"] = _build_module()
    return _cache["nc"]


def _route(x, gate_w, gate_b):
    """Replicates the reference router in numpy fp32. Returns combine [B,E]."""
    pooled = x.mean(axis=(2, 3), dtype=np.float32)
    logits = pooled @ gate_w + gate_b
    z = logits - logits.max(axis=-1, keepdims=True)
    ez = np.exp(z)
    w = ez / ez.sum(axis=-1, keepdims=True)
    topi = np.argsort(-w, axis=-1, kind="stable")[:, :TOPK]
    topw = np.take_along_axis(w, topi, axis=-1)
    topw = topw / (topw.sum(-1, keepdims=True) + 1e-10)
    combine = np.zeros((B, E), np.float32)
    np.put_along_axis(combine, topi, topw, axis=-1)
    return combine


def make_in_maps(x, gate_w, gate_b, expert_w, expert_b):
    x = np.ascontiguousarray(np.asarray(x, np.float32))
    gate_w = np.asarray(gate_w, np.float32)
    gate_b = np.asarray(gate_b, np.float32)
    expert_w = np.asarray(expert_w, np.float32)
    expert_b = np.asarray(expert_b, np.float32)

    combine = _route(x, gate_w, gate_b)                       # [B,E]
    Wc = np.einsum("be,eoikl->boikl", combine, expert_w)      # [B,C,C,3,3]
    bc = combine @ expert_b                                   # [B,C]

    # Padded input images: [B, C, 34*34]
    xp = np.zeros((B, C, PW, PW), np.float32)
    xp[:, :, 1 : H + 1, 1 : W + 1] = x
    xp = xp.reshape(B, C, PP)

    # lhsT layout: WT[b, t, i, o] = Wc[b, o, i, dy, dx]
    WT = Wc.transpose(0, 3, 4, 2, 1).reshape(B, 9, C, C)      # [B, 9, in, out]
    # wa[b, p, t*192+m] = WT[b,t,p,m] for p<128
    wa = np.ascontiguousarray(
        WT[:, :, 0:128, :].transpose(0, 2, 1, 3).reshape(B, 128, 9 * C)
    )
    # K64 weights with duplicated partition halves:
    # wbb[b, p, t*192+m] = WT[b, t, 128 + (p % 64), m]
    wbb = np.ascontiguousarray(
        WT[:, :, 128:192, :].transpose(0, 2, 1, 3).reshape(B, 64, 9 * C)
    )

    import ml_dtypes
    bf16 = ml_dtypes.bfloat16
    xp = xp.astype(bf16)
    wa = wa.astype(bf16)
    wbb = wbb.astype(bf16)

    in_maps = []
    for c in range(NCORES):
        b0 = S * c
        bias = np.zeros((128, 4), np.float32)
        for s in range(S):
            bias[:, s] = bc[b0 + s, 0:128]
            bias[0:64, 2 + s] = bc[b0 + s, 128:192]
        in_maps.append(
            {
                "xp": np.ascontiguousarray(xp[b0 : b0 + S]),
                "wa": np.ascontiguousarray(wa[b0 : b0 + S]),
                "wbb": np.ascontiguousarray(wbb[b0 : b0 + S]),
                "bias": bias,
            }
        )
    return in_maps


def kernel(x, gate_w, gate_b, expert_w, expert_b):
    from concourse.bass_utils import run_bass_kernel_spmd

    nc = get_module()
    in_maps = make_in_maps(x, gate_w, gate_b, expert_w, expert_b)
    res = run_bass_kernel_spmd(nc, in_maps, core_ids=list(range(NCORES)))
    out = np.stack([res.results[c]["out"] for c in range(NCORES)])  # [8,S,C,HWP]
    return out.reshape(B, C, H, W)



# revision 3
# speedup vs baseline: 1.1887x; 1.1887x over previous
"""MoE block (B=16, C=192, H=W=32, E=8, top-2, 3x3 same-conv experts) on 8 trn2 cores.

Strategy (v2 — Winograd):
  - Router + top-2 combine computed on host; conv is linear in weights, so
    each sample gets ONE host-combined 3x3 conv (2 samples per core).
  - F(2x2, 3x3) Winograd: host precomputes the input transform V = B^T d B
    (bf16) and the weight transform U = G g G^T per sample.
  - The output ROW transform (A^T, rows of the 4x4 position grid) is folded
    into PSUM accumulation: for each output-row-parity i and column-position
    v, plane P[i][v] = sum_u A^T[i,u] M[u][v] accumulates directly via
    matmuls whose lhsT carries the +-U signs.
  - K-perfect chunking: the 4 positions x 192 channels = 768 contraction rows
    per (v) are stored as ONE SBUF "super-stack" [128 x 6 chunks x 256 tiles];
    P[0] consumes rows 0..575, P[1] rows 192..767 via partition-offset
    matmuls.  10 matmuls of N=256 per (i, v, M-chunk pair) -> 20480 PE rows
    per sample (vs 36864 for the direct 9-tap conv).
  - Device output COLUMN transform on DVE: Y[i][0] = P0+P1+P2,
    Y[i][1] = P1-P2-P3, with the per-channel bias folded in through
    scalar_tensor_tensor's per-partition scalar operand.
  - Outputs leave as bf16 planes [s, i, o, (j, tile)]; host reassembles and
    upcasts to fp32.
"""

import numpy as np

B, C, H, W = 16, 192, 32, 32
E, TOPK = 8, 2
NCORES = 8
S = B // NCORES          # samples per core
NT = 16                  # output tiles per spatial dim
TILES = NT * NT          # 256 = matmul moving dim
N_WARMUP = 10

_cache = {}

# F(2x2, 3x3) transform matrices
_G = np.array([[1, 0, 0], [0.5, 0.5, 0.5], [0.5, -0.5, 0.5], [0, 0, 1]], np.float32)
_Bt = np.array([[1, 0, -1, 0], [0, 1, 1, 0], [0, -1, 1, 0], [0, 1, 0, -1]], np.float32)

# per-(i, v) matmul chunk plans: (slot in ust tile, part-slice, v-chunk in vstk)
_FULL = (0, 128)
_LO = (0, 64)
_HI = (64, 128)
_PLAN = {
    0: [(0, _FULL, 0), (1, _FULL, 1), (2, _FULL, 2), (3, _FULL, 3), (4, _LO, 4)],
    1: [(4, _HI, 1), (5, _FULL, 2), (6, _FULL, 3), (7, _FULL, 4), (8, _FULL, 5)],
}


def _build_module():
    import concourse.tile as tile
    from concourse import bacc, mybir

    f32 = mybir.dt.float32
    bf16 = mybir.dt.bfloat16
    add = mybir.AluOpType.add
    sub = mybir.AluOpType.subtract

    nc = bacc.Bacc("TRN2", target_bir_lowering=False, debug=False, num_devices=NCORES)
    vstk_d = nc.dram_tensor("vstk", [S, 4, 128, 6, TILES], bf16, kind="ExternalInput")
    ustk_d = nc.dram_tensor("ustk", [S, 4, 128, 9, C], bf16, kind="ExternalInput")
    bias_d = nc.dram_tensor("bias", [128, 2 * S], f32, kind="ExternalInput")
    out_d = nc.dram_tensor("out", [S, 2, C, 2 * TILES], bf16, kind="ExternalOutput")

    with tile.TileContext(nc) as tc:
        with (
            tc.tile_pool(name="vin", bufs=1) as vin,
            tc.tile_pool(name="uin", bufs=1) as uin,
            tc.tile_pool(name="cst", bufs=1) as cst,
            tc.tile_pool(name="psA", bufs=2, space="PSUM") as psA,
            tc.tile_pool(name="psB", bufs=2, space="PSUM") as psB,
            tc.tile_pool(name="tsc", bufs=2) as tsc,
            tc.tile_pool(name="yout", bufs=2) as yout,
        ):
            # PE warmup while input DMAs stream in.
            scr = cst.tile([128, TILES], bf16, name="scr", tag="scr")
            nc.vector.memset(scr[:], 0.0)
            ps_scr = psA.tile([128, 4, TILES], f32, name="ps_scr", tag="psA")
            for _ in range(N_WARMUP):
                nc.tensor.matmul(ps_scr[:, 0, :], scr[:, 0:128], scr[:],
                                 start=True, stop=True, skip_group_check=True)

            bias_t = cst.tile([128, 2 * S], f32, name="bias_t", tag="bias_t")

            VT = {}
            UT = {}

            def emit_input_dmas(s):
                for v in range(4):
                    vt = vin.tile([128, 6, TILES], bf16, name=f"vt{s}{v}",
                                  tag=f"vt{s}{v}")
                    nc.sync.dma_start(vt[:], vstk_d[s, v])
                    VT[(s, v)] = vt
                    ut = uin.tile([128, 9, C], bf16, name=f"ut{s}{v}",
                                  tag=f"ut{s}{v}")
                    eng = nc.scalar if v % 2 == 0 else nc.gpsimd
                    eng.dma_start(ut[:], ustk_d[s, v])
                    UT[(s, v)] = ut
                if s == 0:
                    nc.scalar.dma_start(bias_t[:], bias_d[:])

            def emit_group(s, i):
                pm0 = psA.tile([128, 4, TILES], f32, name=f"pm0_{s}{i}", tag="psA")
                pm1 = psB.tile([64, 4, TILES], f32, name=f"pm1_{s}{i}", tag="psB")
                for v in range(4):
                    vt, ut = VT[(s, v)], UT[(s, v)]
                    plan = _PLAN[i]
                    for idx, (slot, (p0, p1), vc) in enumerate(plan):
                        st, sp = idx == 0, idx == len(plan) - 1
                        nc.tensor.matmul(pm0[:, v, :], ut[p0:p1, slot, 0:128],
                                         vt[p0:p1, vc, :], start=st, stop=sp)
                    for idx, (slot, (p0, p1), vc) in enumerate(plan):
                        st, sp = idx == 0, idx == len(plan) - 1
                        nc.tensor.matmul(pm1[:, v, :], ut[p0:p1, slot, 128:192],
                                         vt[p0:p1, vc, :], start=st, stop=sp)

                # Column transform: Y0 = P0+P1+P2+b ; Y1 = P1-P2-P3+b.
                # ACT evicts P1 with bias folded (DVE may read only one PSUM
                # operand per op), then DVE does the 2-term combines.
                ym0 = yout.tile([128, 2, TILES], bf16, name=f"ym0_{s}{i}", tag="ym0")
                ym1 = yout.tile([64, 2, TILES], bf16, name=f"ym1_{s}{i}", tag="ym1")
                for pm, ym, np_, bcol in ((pm0, ym0, 128, 2 * s),
                                          (pm1, ym1, 64, 2 * s + 1)):
                    p1b = tsc.tile([np_, TILES], bf16, name=f"p1b_{s}{i}_{np_}",
                                   tag=f"p1b{np_}")
                    nc.scalar.activation(
                        p1b[:], pm[:, 1, :],
                        mybir.ActivationFunctionType.Identity,
                        bias=bias_t[0:np_, bcol:bcol + 1], scale=1.0)
                    t0 = tsc.tile([np_, TILES], bf16, name=f"t0_{s}{i}_{np_}",
                                  tag=f"t0{np_}")
                    nc.vector.tensor_tensor(t0[:], p1b[:], pm[:, 0, :], add)
                    nc.vector.tensor_tensor(ym[:, 0, :], t0[:], pm[:, 2, :], add)
                    t1 = tsc.tile([np_, TILES], bf16, name=f"t1_{s}{i}_{np_}",
                                  tag=f"t1{np_}")
                    nc.vector.tensor_tensor(t1[:], p1b[:], pm[:, 2, :], sub)
                    nc.vector.tensor_tensor(ym[:, 1, :], t1[:], pm[:, 3, :], sub)
                nc.gpsimd.dma_start(out_d[s, i, 0:128, :], ym0[:])
                nc.gpsimd.dma_start(out_d[s, i, 128:192, :], ym1[:])

            emit_input_dmas(0)
            emit_input_dmas(1)
            for s in range(S):
                for i in range(2):
                    emit_group(s, i)

    nc.compile()
    return nc


def get_module():
    if "nc" not in _cache:
        _cache["nc"] = _build_module()
    return _cache["nc"]


def _route(x, gate_w, gate_b):
    """Replicates the reference router in numpy fp32. Returns combine [B,E]."""
    pooled = x.mean(axis=(2, 3), dtype=np.float32)
    logits = pooled @ gate_w + gate_b
    z = logits - logits.max(axis=-1, keepdims=True)
    ez = np.exp(z)
    w = ez / ez.sum(axis=-1, keepdims=True)
    topi = np.argsort(-w, axis=-1, kind="stable")[:, :TOPK]
    topw = np.take_along_axis(w, topi, axis=-1)
    topw = topw / (topw.sum(-1, keepdims=True) + 1e-10)
    combine = np.zeros((B, E), np.float32)
    np.put_along_axis(combine, topi, topw, axis=-1)
    return combine


def make_in_maps(x, gate_w, gate_b, expert_w, expert_b):
    import ml_dtypes
    bf16 = ml_dtypes.bfloat16

    x = np.ascontiguousarray(np.asarray(x, np.float32))
    gate_w = np.asarray(gate_w, np.float32)
    gate_b = np.asarray(gate_b, np.float32)
    expert_w = np.asarray(expert_w, np.float32)
    expert_b = np.asarray(expert_b, np.float32)

    combine = _route(x, gate_w, gate_b)                       # [B,E]
    Wc = np.einsum("be,eoikl->boikl", combine, expert_w)      # [B,C,C,3,3]
    bc = combine @ expert_b                                   # [B,C]

    # Weight transform U[b,u,v,c,o] (lhsT layout: K=in-ch, M=out-ch)
    U = np.einsum("uk,bockl,vl->buvco", _G, Wc, _G)

    # Input transform V[b,u,v,c,r,t]
    xp = np.zeros((B, C, H + 2, W + 2), np.float32)
    xp[:, :, 1:H + 1, 1:W + 1] = x
    d = np.empty((B, 4, 4, C, NT, NT), np.float32)
    for u in range(4):
        for v in range(4):
            d[:, u, v] = xp[:, :, u:u + 2 * NT:2, v:v + 2 * NT:2]
    V = np.einsum("uk,bklcrt,vl->buvcrt", _Bt, d, _Bt)

    # V super-stack [b, v, row=(u*C+c), tile] -> [b, v, part, chunk, tile]
    vstk = V.transpose(0, 2, 1, 3, 4, 5).reshape(B, 4, 4 * C, TILES)
    vstk = vstk.reshape(B, 4, 6, 128, TILES).transpose(0, 1, 3, 2, 4)
    vstk = np.ascontiguousarray(vstk.astype(bf16))            # [B,4,128,6,T]

    # U stacks with A^T row-fold signs baked in. ust[b, v, slot, part, o]
    Uall = U.transpose(0, 2, 1, 3, 4).reshape(B, 4, 4 * C, C)  # rows (u, c)
    ust = np.zeros((B, 4, 9, 128, C), np.float32)
    for k in range(4):                       # i=0 chunks 0..3 (+, rows 0..511)
        ust[:, :, k] = Uall[:, :, 128 * k:128 * (k + 1)]
    ust[:, :, 4, 0:64] = Uall[:, :, 512:576]      # i=0 chunk 4 (+)
    ust[:, :, 4, 64:128] = Uall[:, :, 192:256]    # i=1 chunk 1-half (+)
    sgn = np.where(np.arange(4 * C) < 384, 1.0, -1.0)[None, None, :, None]
    Usgn = Uall * sgn                        # i=1 signs: +U1, -U2, -U3
    for k in range(5, 9):                    # i=1 chunks (rows 256..767)
        r0 = 256 + 128 * (k - 5)
        ust[:, :, k] = Usgn[:, :, r0:r0 + 128]
    ust = np.ascontiguousarray(ust.transpose(0, 1, 3, 2, 4).astype(bf16))
    # [B, 4, 128, 9, C]

    in_maps = []
    for c in range(NCORES):
        b0 = S * c
        bias = np.zeros((128, 2 * S), np.float32)
        for s in range(S):
            bias[:, 2 * s] = bc[b0 + s, 0:128]
            bias[0:64, 2 * s + 1] = bc[b0 + s, 128:192]
        in_maps.append({
            "vstk": np.ascontiguousarray(vstk[b0:b0 + S]),
            "ustk": np.ascontiguousarray(ust[b0:b0 + S]),
            "bias": bias,
        })
    return in_maps


def unshard_core(out_arr):
    """[S, 2, C, 2*TILES] bf16 -> [S, C, H, W] fp32 for one core."""
    a = np.asarray(out_arr, np.float32).reshape(S, 2, C, 2, NT, NT)
    full = np.empty((S, C, H, W), np.float32)
    for i in range(2):
        for j in range(2):
            full[:, :, i::2, j::2] = a[:, i, :, j]
    return full


def kernel(x, gate_w, gate_b, expert_w, expert_b):
    from concourse.bass_utils import run_bass_kernel_spmd

    nc = get_module()
    in_maps = make_in_maps(x, gate_w, gate_b, expert_w, expert_b)
    res = run_bass_kernel_spmd(nc, in_maps, core_ids=list(range(NCORES)))
    out = np.concatenate(
        [unshard_core(res.results[c]["out"]) for c in range(NCORES)])
    return out.reshape(B, C, H, W)


# revision 5
# speedup vs baseline: 1.2527x; 1.0539x over previous
"""MoE block (B=16, C=192, H=W=32, E=8, top-2, 3x3 same-conv experts) on 8 trn2 cores.

Strategy (v2 — Winograd):
  - Router + top-2 combine computed on host; conv is linear in weights, so
    each sample gets ONE host-combined 3x3 conv (2 samples per core).
  - F(2x2, 3x3) Winograd: host precomputes the input transform V = B^T d B
    (bf16) and the weight transform U = G g G^T per sample.
  - The output ROW transform (A^T, rows of the 4x4 position grid) is folded
    into PSUM accumulation: for each output-row-parity i and column-position
    v, plane P[i][v] = sum_u A^T[i,u] M[u][v] accumulates directly via
    matmuls whose lhsT carries the +-U signs.
  - K-perfect chunking: the 4 positions x 192 channels = 768 contraction rows
    per (v) are stored as ONE SBUF "super-stack" [128 x 6 chunks x 256 tiles];
    P[0] consumes rows 0..575, P[1] rows 192..767 via partition-offset
    matmuls.  10 matmuls of N=256 per (i, v, M-chunk pair) -> 20480 PE rows
    per sample (vs 36864 for the direct 9-tap conv).
  - Device output COLUMN transform on DVE: Y[i][0] = P0+P1+P2,
    Y[i][1] = P1-P2-P3, with the per-channel bias folded in through
    scalar_tensor_tensor's per-partition scalar operand.
  - Outputs leave as bf16 planes [s, i, o, (j, tile)]; host reassembles and
    upcasts to fp32.
"""

import numpy as np

B, C, H, W = 16, 192, 32, 32
E, TOPK = 8, 2
NCORES = 8
S = B // NCORES          # samples per core
NT = 16                  # output tiles per spatial dim
TILES = NT * NT          # 256 = matmul moving dim
N_WARMUP = 10

_cache = {}

# F(2x2, 3x3) transform matrices
_G = np.array([[1, 0, 0], [0.5, 0.5, 0.5], [0.5, -0.5, 0.5], [0, 0, 1]], np.float32)
_Bt = np.array([[1, 0, -1, 0], [0, 1, 1, 0], [0, -1, 1, 0], [0, 1, 0, -1]], np.float32)

# per-(i, v) matmul chunk plans: (slot in ust tile, part-slice, v-chunk in vstk)
_FULL = (0, 128)
_LO = (0, 64)
_HI = (64, 128)
_PLAN = {
    0: [(0, _FULL, 0), (1, _FULL, 1), (2, _FULL, 2), (3, _FULL, 3), (4, _LO, 4)],
    1: [(4, _HI, 1), (5, _FULL, 2), (6, _FULL, 3), (7, _FULL, 4), (8, _FULL, 5)],
}


def _build_module():
    import concourse.tile as tile
    from concourse import bacc, mybir

    f32 = mybir.dt.float32
    bf16 = mybir.dt.bfloat16
    add = mybir.AluOpType.add
    sub = mybir.AluOpType.subtract

    nc = bacc.Bacc("TRN2", target_bir_lowering=False, debug=False, num_devices=NCORES)
    vstk_d = nc.dram_tensor("vstk", [S, 4, 128, 6, TILES], bf16, kind="ExternalInput")
    ustk_d = nc.dram_tensor("ustk", [S, 4, 128, 9, C], bf16, kind="ExternalInput")
    bias_d = nc.dram_tensor("bias", [128, 2 * S], f32, kind="ExternalInput")
    out_d = nc.dram_tensor("out", [S, 2, C, 2 * TILES], bf16, kind="ExternalOutput")

    with tile.TileContext(nc) as tc:
        with (
            tc.tile_pool(name="vin", bufs=1) as vin,
            tc.tile_pool(name="uin", bufs=1) as uin,
            tc.tile_pool(name="cst", bufs=1) as cst,
            tc.tile_pool(name="psA", bufs=2, space="PSUM") as psA,
            tc.tile_pool(name="psB", bufs=2, space="PSUM") as psB,
            tc.tile_pool(name="tsc", bufs=2) as tsc,
            tc.tile_pool(name="yout", bufs=2) as yout,
        ):
            # PE warmup while input DMAs stream in.
            scr = cst.tile([128, TILES], bf16, name="scr", tag="scr")
            nc.vector.memset(scr[:], 0.0)
            ps_scr = psA.tile([128, 4, TILES], f32, name="ps_scr", tag="psA")
            for _ in range(N_WARMUP):
                nc.tensor.matmul(ps_scr[:, 0, :], scr[:, 0:128], scr[:],
                                 start=True, stop=True, skip_group_check=True)

            bias_t = cst.tile([128, 2 * S], f32, name="bias_t", tag="bias_t")

            VT = {}
            UT = {}

            def emit_input_dmas(s):
                for v in range(4):
                    vt = vin.tile([128, 6, TILES], bf16, name=f"vt{s}{v}",
                                  tag=f"vt{s}{v}")
                    if s == 0 and v == 0:
                        # split so the very first matmul's data lands early
                        nc.sync.dma_start(vt[:, 0:1, :], vstk_d[s, v, :, 0:1])
                        nc.sync.dma_start(vt[:, 1:6, :], vstk_d[s, v, :, 1:6])
                    else:
                        nc.sync.dma_start(vt[:], vstk_d[s, v])
                    VT[(s, v)] = vt
                    ut = uin.tile([128, 9, C], bf16, name=f"ut{s}{v}",
                                  tag=f"ut{s}{v}")
                    eng = nc.scalar if v % 2 == 0 else nc.gpsimd
                    if s == 0 and v == 0:
                        eng.dma_start(ut[:, 0:3, :], ustk_d[s, v, :, 0:3])
                        eng.dma_start(ut[:, 3:9, :], ustk_d[s, v, :, 3:9])
                    else:
                        eng.dma_start(ut[:], ustk_d[s, v])
                    UT[(s, v)] = ut
                if s == 0:
                    nc.scalar.dma_start(bias_t[:], bias_d[:])

            def emit_sample(s):
                """Matmuls v-major with i0/i1 interleaved (matches DMA arrival
                order); the column transform Y0 = P0+P1+P2+b, Y1 = P1-P2-P3+b
                is pipelined into the v loop so only y1 + out-DMA trail the
                last matmul.  ACT evicts P1 with bias folded (DVE may read
                only one PSUM operand per op)."""
                pm0 = {i: psA.tile([128, 4, TILES], f32, name=f"pm0_{s}{i}",
                                   tag="psA") for i in range(2)}
                pm1 = {i: psB.tile([64, 4, TILES], f32, name=f"pm1_{s}{i}",
                                   tag="psB") for i in range(2)}
                ym0 = {i: yout.tile([128, 2, TILES], bf16, name=f"ym0_{s}{i}",
                                    tag=f"ym0_{i}") for i in range(2)}
                ym1 = {i: yout.tile([64, 2, TILES], bf16, name=f"ym1_{s}{i}",
                                    tag=f"ym1_{i}") for i in range(2)}
                p1b, t0, t1 = {}, {}, {}

                def quads(i):
                    # (pm, ym, n_parts, bias col) for both M-chunks of group i
                    return ((pm0[i], ym0[i], 128, 2 * s),
                            (pm1[i], ym1[i], 64, 2 * s + 1))

                for v in range(4):
                    vt, ut = VT[(s, v)], UT[(s, v)]
                    for i in range(2):
                        plan = _PLAN[i]
                        for mlo, mhi, pm in ((0, 128, pm0[i]), (128, 192, pm1[i])):
                            for idx, (slot, (p0, p1), vc) in enumerate(plan):
                                nc.tensor.matmul(
                                    pm[:, v, :], ut[p0:p1, slot, mlo:mhi],
                                    vt[p0:p1, vc, :],
                                    start=(idx == 0), stop=(idx == len(plan) - 1))
                    if v == 1:
                        for i in range(2):
                            for k, (pm, ym, np_, bcol) in enumerate(quads(i)):
                                pb = tsc.tile([np_, TILES], bf16,
                                              name=f"p1b_{s}{i}{k}", tag=f"p1b{i}{k}")
                                nc.scalar.activation(
                                    pb[:], pm[:, 1, :],
                                    mybir.ActivationFunctionType.Identity,
                                    bias=bias_t[0:np_, bcol:bcol + 1], scale=1.0)
                                p1b[(i, k)] = pb
                                tt = tsc.tile([np_, TILES], bf16,
                                              name=f"t0_{s}{i}{k}", tag=f"t0{i}{k}")
                                nc.vector.tensor_tensor(tt[:], pb[:], pm[:, 0, :], add)
                                t0[(i, k)] = tt
                    elif v == 2:
                        for i in range(2):
                            for k, (pm, ym, np_, bcol) in enumerate(quads(i)):
                                nc.vector.tensor_tensor(
                                    ym[:, 0, :], t0[(i, k)][:], pm[:, 2, :], add)
                                tt = tsc.tile([np_, TILES], bf16,
                                              name=f"t1_{s}{i}{k}", tag=f"t1{i}{k}")
                                nc.vector.tensor_tensor(
                                    tt[:], p1b[(i, k)][:], pm[:, 2, :], sub)
                                t1[(i, k)] = tt
                    elif v == 3:
                        for i in range(2):
                            for k, (pm, ym, np_, bcol) in enumerate(quads(i)):
                                nc.vector.tensor_tensor(
                                    ym[:, 1, :], t1[(i, k)][:], pm[:, 3, :], sub)
                            nc.sync.dma_start(out_d[s, i, 0:128, :], ym0[i][:])
                            nc.sync.dma_start(out_d[s, i, 128:192, :], ym1[i][:])

            emit_input_dmas(0)
            emit_input_dmas(1)
            for s in range(S):
                emit_sample(s)

    nc.compile()
    return nc


def get_module():
    if "nc" not in _cache:
        _cache["nc"] = _build_module()
    return _cache["nc"]


def _route(x, gate_w, gate_b):
    """Replicates the reference router in numpy fp32. Returns combine [B,E]."""
    pooled = x.mean(axis=(2, 3), dtype=np.float32)
    logits = pooled @ gate_w + gate_b
    z = logits - logits.max(axis=-1, keepdims=True)
    ez = np.exp(z)
    w = ez / ez.sum(axis=-1, keepdims=True)
    topi = np.argsort(-w, axis=-1, kind="stable")[:, :TOPK]
    topw = np.take_along_axis(w, topi, axis=-1)
    topw = topw / (topw.sum(-1, keepdims=True) + 1e-10)
    combine = np.zeros((B, E), np.float32)
    np.put_along_axis(combine, topi, topw, axis=-1)
    return combine


def make_in_maps(x, gate_w, gate_b, expert_w, expert_b):
    import ml_dtypes
    bf16 = ml_dtypes.bfloat16

    x = np.ascontiguousarray(np.asarray(x, np.float32))
    gate_w = np.asarray(gate_w, np.float32)
    gate_b = np.asarray(gate_b, np.float32)
    expert_w = np.asarray(expert_w, np.float32)
    expert_b = np.asarray(expert_b, np.float32)

    combine = _route(x, gate_w, gate_b)                       # [B,E]
    Wc = np.einsum("be,eoikl->boikl", combine, expert_w)      # [B,C,C,3,3]
    bc = combine @ expert_b                                   # [B,C]

    # Weight transform U[b,u,v,c,o] (lhsT layout: K=in-ch, M=out-ch)
    U = np.einsum("uk,bockl,vl->buvco", _G, Wc, _G)

    # Input transform V[b,u,v,c,r,t]
    xp = np.zeros((B, C, H + 2, W + 2), np.float32)
    xp[:, :, 1:H + 1, 1:W + 1] = x
    d = np.empty((B, 4, 4, C, NT, NT), np.float32)
    for u in range(4):
        for v in range(4):
            d[:, u, v] = xp[:, :, u:u + 2 * NT:2, v:v + 2 * NT:2]
    V = np.einsum("uk,bklcrt,vl->buvcrt", _Bt, d, _Bt)

    # V super-stack [b, v, row=(u*C+c), tile] -> [b, v, part, chunk, tile]
    vstk = V.transpose(0, 2, 1, 3, 4, 5).reshape(B, 4, 4 * C, TILES)
    vstk = vstk.reshape(B, 4, 6, 128, TILES).transpose(0, 1, 3, 2, 4)
    vstk = np.ascontiguousarray(vstk.astype(bf16))            # [B,4,128,6,T]

    # U stacks with A^T row-fold signs baked in. ust[b, v, slot, part, o]
    Uall = U.transpose(0, 2, 1, 3, 4).reshape(B, 4, 4 * C, C)  # rows (u, c)
    ust = np.zeros((B, 4, 9, 128, C), np.float32)
    for k in range(4):                       # i=0 chunks 0..3 (+, rows 0..511)
        ust[:, :, k] = Uall[:, :, 128 * k:128 * (k + 1)]
    ust[:, :, 4, 0:64] = Uall[:, :, 512:576]      # i=0 chunk 4 (+)
    ust[:, :, 4, 64:128] = Uall[:, :, 192:256]    # i=1 chunk 1-half (+)
    sgn = np.where(np.arange(4 * C) < 384, 1.0, -1.0)[None, None, :, None]
    Usgn = Uall * sgn                        # i=1 signs: +U1, -U2, -U3
    for k in range(5, 9):                    # i=1 chunks (rows 256..767)
        r0 = 256 + 128 * (k - 5)
        ust[:, :, k] = Usgn[:, :, r0:r0 + 128]
    ust = np.ascontiguousarray(ust.transpose(0, 1, 3, 2, 4).astype(bf16))
    # [B, 4, 128, 9, C]

    in_maps = []
    for c in range(NCORES):
        b0 = S * c
        bias = np.zeros((128, 2 * S), np.float32)
        for s in range(S):
            bias[:, 2 * s] = bc[b0 + s, 0:128]
            bias[0:64, 2 * s + 1] = bc[b0 + s, 128:192]
        in_maps.append({
            "vstk": np.ascontiguousarray(vstk[b0:b0 + S]),
            "ustk": np.ascontiguousarray(ust[b0:b0 + S]),
            "bias": bias,
        })
    return in_maps


def unshard_core(out_arr):
    """[S, 2, C, 2*TILES] bf16 -> [S, C, H, W] fp32 for one core."""
    a = np.asarray(out_arr, np.float32).reshape(S, 2, C, 2, NT, NT)
    full = np.empty((S, C, H, W), np.float32)
    for i in range(2):
        for j in range(2):
            full[:, :, i::2, j::2] = a[:, i, :, j]
    return full


def kernel(x, gate_w, gate_b, expert_w, expert_b):
    from concourse.bass_utils import run_bass_kernel_spmd

    nc = get_module()
    in_maps = make_in_maps(x, gate_w, gate_b, expert_w, expert_b)
    res = run_bass_kernel_spmd(nc, in_maps, core_ids=list(range(NCORES)))
    out = np.concatenate(
        [unshard_core(res.results[c]["out"]) for c in range(NCORES)])
    return out.reshape(B, C, H, W)


# revision 7
# speedup vs baseline: 1.3730x; 1.0960x over previous
"""MoE block (B=16, C=192, H=W=32, E=8, top-2, 3x3 same-conv experts) on 8 trn2 cores.

Strategy (v2 — Winograd):
  - Router + top-2 combine computed on host; conv is linear in weights, so
    each sample gets ONE host-combined 3x3 conv (2 samples per core).
  - F(2x2, 3x3) Winograd: host precomputes the input transform V = B^T d B
    (bf16) and the weight transform U = G g G^T per sample.
  - The output ROW transform (A^T, rows of the 4x4 position grid) is folded
    into PSUM accumulation: for each output-row-parity i and column-position
    v, plane P[i][v] = sum_u A^T[i,u] M[u][v] accumulates directly via
    matmuls whose lhsT carries the +-U signs.
  - K-perfect chunking: the 4 positions x 192 channels = 768 contraction rows
    per (v) are stored as ONE SBUF "super-stack" [128 x 6 chunks x 256 tiles];
    P[0] consumes rows 0..575, P[1] rows 192..767 via partition-offset
    matmuls.  10 matmuls of N=256 per (i, v, M-chunk pair) -> 20480 PE rows
    per sample (vs 36864 for the direct 9-tap conv).
  - Device output COLUMN transform on DVE: Y[i][0] = P0+P1+P2,
    Y[i][1] = P1-P2-P3, with the per-channel bias folded in through
    scalar_tensor_tensor's per-partition scalar operand.
  - Outputs leave as bf16 planes [s, i, o, (j, tile)]; host reassembles and
    upcasts to fp32.
"""

import numpy as np

B, C, H, W = 16, 192, 32, 32
E, TOPK = 8, 2
NCORES = 8
S = B // NCORES          # samples per core
NT = 16                  # output tiles per spatial dim
TILES = NT * NT          # 256 = matmul moving dim
N_WARMUP = 10

_cache = {}

# F(2x2, 3x3) transform matrices
_G = np.array([[1, 0, 0], [0.5, 0.5, 0.5], [0.5, -0.5, 0.5], [0, 0, 1]], np.float32)
_Bt = np.array([[1, 0, -1, 0], [0, 1, 1, 0], [0, -1, 1, 0], [0, 1, 0, -1]], np.float32)

# per-(i, v) matmul chunk plans: (slot in ust tile, part-slice, v-chunk in vstk)
_FULL = (0, 128)
_LO = (0, 64)
_HI = (64, 128)
_PLAN = {
    0: [(0, _FULL, 0), (1, _FULL, 1), (2, _FULL, 2), (3, _FULL, 3), (4, _LO, 4)],
    1: [(4, _HI, 1), (5, _FULL, 2), (6, _FULL, 3), (7, _FULL, 4), (8, _FULL, 5)],
}


def _build_module():
    import concourse.tile as tile
    from concourse import bacc, mybir

    f32 = mybir.dt.float32
    bf16 = mybir.dt.bfloat16
    add = mybir.AluOpType.add
    sub = mybir.AluOpType.subtract

    nc = bacc.Bacc("TRN2", target_bir_lowering=False, debug=False, num_devices=NCORES)
    vstk_d = nc.dram_tensor("vstk", [S, 4, 128, 6, TILES], bf16, kind="ExternalInput")
    ustk_d = nc.dram_tensor("ustk", [S, 4, 128, 9, C], bf16, kind="ExternalInput")
    bias_d = nc.dram_tensor("bias", [128, 2 * S], f32, kind="ExternalInput")
    out_d = nc.dram_tensor("out", [S, 2, C, 2 * TILES], bf16, kind="ExternalOutput")

    with tile.TileContext(nc) as tc:
        with (
            tc.tile_pool(name="vin", bufs=1) as vin,
            tc.tile_pool(name="uin", bufs=1) as uin,
            tc.tile_pool(name="cst", bufs=1) as cst,
            tc.tile_pool(name="psA", bufs=2, space="PSUM") as psA,
            tc.tile_pool(name="psB", bufs=2, space="PSUM") as psB,
            tc.tile_pool(name="tsc", bufs=2) as tsc,
            tc.tile_pool(name="yout", bufs=2) as yout,
        ):
            # PE warmup while input DMAs stream in.
            scr = cst.tile([128, TILES], bf16, name="scr", tag="scr")
            nc.vector.memset(scr[:], 0.0)
            ps_scr = psA.tile([128, 4, TILES], f32, name="ps_scr", tag="psA")
            for _ in range(N_WARMUP):
                nc.tensor.matmul(ps_scr[:, 0, :], scr[:, 0:128], scr[:],
                                 start=True, stop=True, skip_group_check=True)

            bias_t = cst.tile([128, 2 * S], f32, name="bias_t", tag="bias_t")

            VT = {}
            UT = {}

            def emit_input_dmas(s):
                # All input DMAs on ONE queue (sync) in exact consumption
                # order, so the serialized DMA-engine track never runs a
                # far-future transfer while a near-term one waits.
                for v in range(4):
                    vt = vin.tile([128, 6, TILES], bf16, name=f"vt{s}{v}",
                                  tag=f"vt{s}{v}")
                    ut = uin.tile([128, 9, C], bf16, name=f"ut{s}{v}",
                                  tag=f"ut{s}{v}")
                    if s == 0 and v == 0:
                        # split so the very first matmul's data lands early
                        nc.sync.dma_start(vt[:, 0:1, :], vstk_d[s, v, :, 0:1])
                        nc.sync.dma_start(ut[:, 0:3, :], ustk_d[s, v, :, 0:3])
                        nc.sync.dma_start(vt[:, 1:6, :], vstk_d[s, v, :, 1:6])
                        nc.sync.dma_start(ut[:, 3:9, :], ustk_d[s, v, :, 3:9])
                    else:
                        nc.sync.dma_start(vt[:], vstk_d[s, v])
                        nc.sync.dma_start(ut[:], ustk_d[s, v])
                    VT[(s, v)] = vt
                    UT[(s, v)] = ut
                if s == 0:
                    nc.scalar.dma_start(bias_t[:], bias_d[:])

            def emit_sample(s):
                """Matmuls v-major with i0/i1 interleaved (matches DMA arrival
                order); the column transform Y0 = P0+P1+P2+b, Y1 = P1-P2-P3+b
                is pipelined into the v loop so only y1 + out-DMA trail the
                last matmul.  ACT evicts P1 with bias folded (DVE may read
                only one PSUM operand per op)."""
                pm0 = {i: psA.tile([128, 4, TILES], f32, name=f"pm0_{s}{i}",
                                   tag="psA") for i in range(2)}
                pm1 = {i: psB.tile([64, 4, TILES], f32, name=f"pm1_{s}{i}",
                                   tag="psB") for i in range(2)}
                ym0 = {i: yout.tile([128, 2, TILES], bf16, name=f"ym0_{s}{i}",
                                    tag=f"ym0_{i}") for i in range(2)}
                ym1 = {i: yout.tile([64, 2, TILES], bf16, name=f"ym1_{s}{i}",
                                    tag=f"ym1_{i}") for i in range(2)}
                p1b, t0, t1 = {}, {}, {}

                def quads(i):
                    # (pm, ym, n_parts, bias col) for both M-chunks of group i
                    return ((pm0[i], ym0[i], 128, 2 * s),
                            (pm1[i], ym1[i], 64, 2 * s + 1))

                for v in range(4):
                    vt, ut = VT[(s, v)], UT[(s, v)]
                    for i in range(2):
                        plan = _PLAN[i]
                        for mlo, mhi, pm in ((0, 128, pm0[i]), (128, 192, pm1[i])):
                            for idx, (slot, (p0, p1), vc) in enumerate(plan):
                                nc.tensor.matmul(
                                    pm[:, v, :], ut[p0:p1, slot, mlo:mhi],
                                    vt[p0:p1, vc, :],
                                    start=(idx == 0), stop=(idx == len(plan) - 1))
                    if v == 1:
                        for i in range(2):
                            for k, (pm, ym, np_, bcol) in enumerate(quads(i)):
                                pb = tsc.tile([np_, TILES], bf16,
                                              name=f"p1b_{s}{i}{k}", tag=f"p1b{i}{k}")
                                nc.scalar.activation(
                                    pb[:], pm[:, 1, :],
                                    mybir.ActivationFunctionType.Identity,
                                    bias=bias_t[0:np_, bcol:bcol + 1], scale=1.0)
                                p1b[(i, k)] = pb
                                tt = tsc.tile([np_, TILES], bf16,
                                              name=f"t0_{s}{i}{k}", tag=f"t0{i}{k}")
                                nc.vector.tensor_tensor(tt[:], pb[:], pm[:, 0, :], add)
                                t0[(i, k)] = tt
                    elif v == 2:
                        for i in range(2):
                            for k, (pm, ym, np_, bcol) in enumerate(quads(i)):
                                nc.vector.tensor_tensor(
                                    ym[:, 0, :], t0[(i, k)][:], pm[:, 2, :], add)
                                tt = tsc.tile([np_, TILES], bf16,
                                              name=f"t1_{s}{i}{k}", tag=f"t1{i}{k}")
                                nc.vector.tensor_tensor(
                                    tt[:], p1b[(i, k)][:], pm[:, 2, :], sub)
                                t1[(i, k)] = tt
                    elif v == 3:
                        last = s == S - 1
                        for i in range(2):
                            for k, (pm, ym, np_, bcol) in enumerate(quads(i)):
                                nc.vector.tensor_tensor(
                                    ym[:, 1, :], t1[(i, k)][:], pm[:, 3, :], sub)
                            if last and i == 1:
                                # split per output column so only the tiny
                                # j=1 halves trail the final matmul/DVE op
                                nc.scalar.dma_start(out_d[s, i, 0:128, 0:TILES],
                                                    ym0[i][:, 0, :])
                                nc.scalar.dma_start(out_d[s, i, 128:192, 0:TILES],
                                                    ym1[i][:, 0, :])
                                nc.scalar.dma_start(
                                    out_d[s, i, 0:128, TILES:2 * TILES],
                                    ym0[i][:, 1, :])
                                nc.scalar.dma_start(
                                    out_d[s, i, 128:192, TILES:2 * TILES],
                                    ym1[i][:, 1, :])
                            else:
                                nc.scalar.dma_start(out_d[s, i, 0:128, :], ym0[i][:])
                                nc.scalar.dma_start(out_d[s, i, 128:192, :], ym1[i][:])

            emit_input_dmas(0)
            emit_input_dmas(1)
            for s in range(S):
                emit_sample(s)

    nc.compile()
    return nc


def get_module():
    if "nc" not in _cache:
        _cache["nc"] = _build_module()
    return _cache["nc"]


def _route(x, gate_w, gate_b):
    """Replicates the reference router in numpy fp32. Returns combine [B,E]."""
    pooled = x.mean(axis=(2, 3), dtype=np.float32)
    logits = pooled @ gate_w + gate_b
    z = logits - logits.max(axis=-1, keepdims=True)
    ez = np.exp(z)
    w = ez / ez.sum(axis=-1, keepdims=True)
    topi = np.argsort(-w, axis=-1, kind="stable")[:, :TOPK]
    topw = np.take_along_axis(w, topi, axis=-1)
    topw = topw / (topw.sum(-1, keepdims=True) + 1e-10)
    combine = np.zeros((B, E), np.float32)
    np.put_along_axis(combine, topi, topw, axis=-1)
    return combine


def make_in_maps(x, gate_w, gate_b, expert_w, expert_b):
    import ml_dtypes
    bf16 = ml_dtypes.bfloat16

    x = np.ascontiguousarray(np.asarray(x, np.float32))
    gate_w = np.asarray(gate_w, np.float32)
    gate_b = np.asarray(gate_b, np.float32)
    expert_w = np.asarray(expert_w, np.float32)
    expert_b = np.asarray(expert_b, np.float32)

    combine = _route(x, gate_w, gate_b)                       # [B,E]
    Wc = np.einsum("be,eoikl->boikl", combine, expert_w)      # [B,C,C,3,3]
    bc = combine @ expert_b                                   # [B,C]

    # Weight transform U[b,u,v,c,o] (lhsT layout: K=in-ch, M=out-ch)
    U = np.einsum("uk,bockl,vl->buvco", _G, Wc, _G)

    # Input transform V[b,u,v,c,r,t]
    xp = np.zeros((B, C, H + 2, W + 2), np.float32)
    xp[:, :, 1:H + 1, 1:W + 1] = x
    d = np.empty((B, 4, 4, C, NT, NT), np.float32)
    for u in range(4):
        for v in range(4):
            d[:, u, v] = xp[:, :, u:u + 2 * NT:2, v:v + 2 * NT:2]
    V = np.einsum("uk,bklcrt,vl->buvcrt", _Bt, d, _Bt)

    # V super-stack [b, v, row=(u*C+c), tile] -> [b, v, part, chunk, tile]
    vstk = V.transpose(0, 2, 1, 3, 4, 5).reshape(B, 4, 4 * C, TILES)
    vstk = vstk.reshape(B, 4, 6, 128, TILES).transpose(0, 1, 3, 2, 4)
    vstk = np.ascontiguousarray(vstk.astype(bf16))            # [B,4,128,6,T]

    # U stacks with A^T row-fold signs baked in. ust[b, v, slot, part, o]
    Uall = U.transpose(0, 2, 1, 3, 4).reshape(B, 4, 4 * C, C)  # rows (u, c)
    ust = np.zeros((B, 4, 9, 128, C), np.float32)
    for k in range(4):                       # i=0 chunks 0..3 (+, rows 0..511)
        ust[:, :, k] = Uall[:, :, 128 * k:128 * (k + 1)]
    ust[:, :, 4, 0:64] = Uall[:, :, 512:576]      # i=0 chunk 4 (+)
    ust[:, :, 4, 64:128] = Uall[:, :, 192:256]    # i=1 chunk 1-half (+)
    sgn = np.where(np.arange(4 * C) < 384, 1.0, -1.0)[None, None, :, None]
    Usgn = Uall * sgn                        # i=1 signs: +U1, -U2, -U3
    for k in range(5, 9):                    # i=1 chunks (rows 256..767)
        r0 = 256 + 128 * (k - 5)
        ust[:, :, k] = Usgn[:, :, r0:r0 + 128]
    ust = np.ascontiguousarray(ust.transpose(0, 1, 3, 2, 4).astype(bf16))
    # [B, 4, 128, 9, C]

    in_maps = []
    for c in range(NCORES):
        b0 = S * c
        bias = np.zeros((128, 2 * S), np.float32)
        for s in range(S):
            bias[:, 2 * s] = bc[b0 + s, 0:128]
            bias[0:64, 2 * s + 1] = bc[b0 + s, 128:192]
        in_maps.append({
            "vstk": np.ascontiguousarray(vstk[b0:b0 + S]),
            "ustk": np.ascontiguousarray(ust[b0:b0 + S]),
            "bias": bias,
        })
    return in_maps


def unshard_core(out_arr):
    """[S, 2, C, 2*TILES] bf16 -> [S, C, H, W] fp32 for one core."""
    a = np.asarray(out_arr, np.float32).reshape(S, 2, C, 2, NT, NT)
    full = np.empty((S, C, H, W), np.float32)
    for i in range(2):
        for j in range(2):
            full[:, :, i::2, j::2] = a[:, i, :, j]
    return full


def kernel(x, gate_w, gate_b, expert_w, expert_b):
    from concourse.bass_utils import run_bass_kernel_spmd

    nc = get_module()
    in_maps = make_in_maps(x, gate_w, gate_b, expert_w, expert_b)
    res = run_bass_kernel_spmd(nc, in_maps, core_ids=list(range(NCORES)))
    out = np.concatenate(
        [unshard_core(res.results[c]["out"]) for c in range(NCORES)])
    return out.reshape(B, C, H, W)


# revision 10
# speedup vs baseline: 1.4131x; 1.0292x over previous
"""MoE block (B=16, C=192, H=W=32, E=8, top-2, 3x3 same-conv experts) on 8 trn2 cores.

Strategy (v2 — Winograd):
  - Router + top-2 combine computed on host; conv is linear in weights, so
    each sample gets ONE host-combined 3x3 conv (2 samples per core).
  - F(2x2, 3x3) Winograd: host precomputes the input transform V = B^T d B
    (bf16) and the weight transform U = G g G^T per sample.
  - The output ROW transform (A^T, rows of the 4x4 position grid) is folded
    into PSUM accumulation: for each output-row-parity i and column-position
    v, plane P[i][v] = sum_u A^T[i,u] M[u][v] accumulates directly via
    matmuls whose lhsT carries the +-U signs.
  - K-perfect chunking: the 4 positions x 192 channels = 768 contraction rows
    per (v) are stored as ONE SBUF "super-stack" [128 x 6 chunks x 256 tiles];
    P[0] consumes rows 0..575, P[1] rows 192..767 via partition-offset
    matmuls.  10 matmuls of N=256 per (i, v, M-chunk pair) -> 20480 PE rows
    per sample (vs 36864 for the direct 9-tap conv).
  - Device output COLUMN transform on DVE: Y[i][0] = P0+P1+P2,
    Y[i][1] = P1-P2-P3, with the per-channel bias folded in through
    scalar_tensor_tensor's per-partition scalar operand.
  - Outputs leave as bf16 planes [s, i, o, (j, tile)]; host reassembles and
    upcasts to fp32.
"""

import numpy as np

B, C, H, W = 16, 192, 32, 32
E, TOPK = 8, 2
NCORES = 8
S = B // NCORES          # samples per core
NT = 16                  # output tiles per spatial dim
TILES = NT * NT          # 256 = matmul moving dim
N_WARMUP = 10

_cache = {}

# F(2x2, 3x3) transform matrices
_G = np.array([[1, 0, 0], [0.5, 0.5, 0.5], [0.5, -0.5, 0.5], [0, 0, 1]], np.float32)
_Bt = np.array([[1, 0, -1, 0], [0, 1, 1, 0], [0, -1, 1, 0], [0, 1, 0, -1]], np.float32)

# per-(i, v) matmul chunk plans: (slot in ust tile, part-slice, v-chunk in vstk)
# U stack layout (8 slots of 128 rows): [U0, U1, U2 | U2hi, pad | -U2, -U3]
# rows 0..575 = +U0,+U1,+U2 (i=0); slot4[64:] pad; slots 5..7 = -U2,-U3
# re-aligned so each slot k pairs with V super-stack chunk at the same
# partition offsets.
_FULL = (0, 128)
_LO = (0, 64)
_HI = (64, 128)
_PLAN = {
    0: [(0, _FULL, 0), (1, _FULL, 1), (2, _FULL, 2), (3, _FULL, 3), (4, _LO, 4)],
    1: [(1, _HI, 1), (2, _FULL, 2), (5, _FULL, 3), (6, _FULL, 4), (7, _FULL, 5)],
}


def _build_module():
    import concourse.tile as tile
    from concourse import bacc, mybir

    f32 = mybir.dt.float32
    bf16 = mybir.dt.bfloat16
    add = mybir.AluOpType.add
    sub = mybir.AluOpType.subtract

    nc = bacc.Bacc("TRN2", target_bir_lowering=False, debug=False, num_devices=NCORES)
    vstk_d = nc.dram_tensor("vstk", [S, 4, 128, 6, TILES], bf16, kind="ExternalInput")
    ustk_d = nc.dram_tensor("ustk", [S, 4, 128, 8, C], bf16, kind="ExternalInput")
    bias_d = nc.dram_tensor("bias", [128, 2 * S], f32, kind="ExternalInput")
    out_d = nc.dram_tensor("out", [S, 2, C, 2 * TILES], bf16, kind="ExternalOutput")

    with tile.TileContext(nc) as tc:
        with (
            tc.tile_pool(name="vin", bufs=1) as vin,
            tc.tile_pool(name="uin", bufs=1) as uin,
            tc.tile_pool(name="cst", bufs=1) as cst,
            tc.tile_pool(name="psA", bufs=2, space="PSUM") as psA,
            tc.tile_pool(name="psB", bufs=2, space="PSUM") as psB,
            tc.tile_pool(name="tsc", bufs=2) as tsc,
            tc.tile_pool(name="yout", bufs=2) as yout,
        ):
            # PE warmup while input DMAs stream in.
            scr = cst.tile([128, TILES], bf16, name="scr", tag="scr")
            nc.vector.memset(scr[:], 0.0)
            ps_scr = psA.tile([128, 4, TILES], f32, name="ps_scr", tag="psA")
            for _ in range(N_WARMUP):
                nc.tensor.matmul(ps_scr[:, 0, :], scr[:, 0:128], scr[:],
                                 start=True, stop=True, skip_group_check=True)

            bias_t = cst.tile([128, 2 * S], f32, name="bias_t", tag="bias_t")

            VT = {}
            UT = {}

            def emit_input_dmas(s):
                # All input DMAs on ONE queue (sync) in exact consumption
                # order, so the serialized DMA-engine track never runs a
                # far-future transfer while a near-term one waits.
                for v in range(4):
                    vt = vin.tile([128, 6, TILES], bf16, name=f"vt{s}{v}",
                                  tag=f"vt{s}{v}")
                    ut = uin.tile([128, 8, C], bf16, name=f"ut{s}{v}",
                                  tag=f"ut{s}{v}")
                    if s == 0 and v == 0:
                        # split so the very first matmul's data lands early
                        nc.sync.dma_start(vt[:, 0:1, :], vstk_d[s, v, :, 0:1])
                        nc.sync.dma_start(ut[:, 0:3, :], ustk_d[s, v, :, 0:3])
                        nc.sync.dma_start(vt[:, 1:6, :], vstk_d[s, v, :, 1:6])
                        nc.sync.dma_start(ut[:, 3:8, :], ustk_d[s, v, :, 3:8])
                    else:
                        nc.sync.dma_start(vt[:], vstk_d[s, v])
                        nc.sync.dma_start(ut[:], ustk_d[s, v])
                    VT[(s, v)] = vt
                    UT[(s, v)] = ut
                if s == 0:
                    nc.scalar.dma_start(bias_t[:], bias_d[:])

            def emit_sample(s):
                """Matmuls v-major with i0/i1 interleaved (matches DMA arrival
                order); the column transform Y0 = P0+P1+P2+b, Y1 = P1-P2-P3+b
                is pipelined into the v loop so only y1 + out-DMA trail the
                last matmul.  ACT evicts P1 with bias folded (DVE may read
                only one PSUM operand per op)."""
                pm0 = {i: psA.tile([128, 4, TILES], f32, name=f"pm0_{s}{i}",
                                   tag="psA") for i in range(2)}
                pm1 = {i: psB.tile([64, 4, TILES], f32, name=f"pm1_{s}{i}",
                                   tag="psB") for i in range(2)}
                ym0 = {i: yout.tile([128, 2, TILES], bf16, name=f"ym0_{s}{i}",
                                    tag=f"ym0_{i}") for i in range(2)}
                ym1 = {i: yout.tile([64, 2, TILES], bf16, name=f"ym1_{s}{i}",
                                    tag=f"ym1_{i}") for i in range(2)}
                p1b, t0, t1 = {}, {}, {}

                def quads(i):
                    # (pm, ym, n_parts, bias col) for both M-chunks of group i
                    return ((pm0[i], ym0[i], 128, 2 * s),
                            (pm1[i], ym1[i], 64, 2 * s + 1))

                for v in range(4):
                    vt, ut = VT[(s, v)], UT[(s, v)]
                    for i in range(2):
                        plan = _PLAN[i]
                        for mlo, mhi, pm in ((0, 128, pm0[i]), (128, 192, pm1[i])):
                            for idx, (slot, (p0, p1), vc) in enumerate(plan):
                                nc.tensor.matmul(
                                    pm[:, v, :], ut[p0:p1, slot, mlo:mhi],
                                    vt[p0:p1, vc, :],
                                    start=(idx == 0), stop=(idx == len(plan) - 1))
                    if v == 1:
                        for i in range(2):
                            for k, (pm, ym, np_, bcol) in enumerate(quads(i)):
                                pb = tsc.tile([np_, TILES], bf16,
                                              name=f"p1b_{s}{i}{k}", tag=f"p1b{i}{k}")
                                nc.scalar.activation(
                                    pb[:], pm[:, 1, :],
                                    mybir.ActivationFunctionType.Identity,
                                    bias=bias_t[0:np_, bcol:bcol + 1], scale=1.0)
                                p1b[(i, k)] = pb
                                tt = tsc.tile([np_, TILES], bf16,
                                              name=f"t0_{s}{i}{k}", tag=f"t0{i}{k}")
                                nc.vector.tensor_tensor(tt[:], pb[:], pm[:, 0, :], add)
                                t0[(i, k)] = tt
                    elif v == 2:
                        last = s == S - 1
                        for i in range(2):
                            for k, (pm, ym, np_, bcol) in enumerate(quads(i)):
                                nc.vector.tensor_tensor(
                                    ym[:, 0, :], t0[(i, k)][:], pm[:, 2, :], add)
                                tt = tsc.tile([np_, TILES], bf16,
                                              name=f"t1_{s}{i}{k}", tag=f"t1{i}{k}")
                                nc.vector.tensor_tensor(
                                    tt[:], p1b[(i, k)][:], pm[:, 2, :], sub)
                                t1[(i, k)] = tt
                            if last and i == 1:
                                # ship the j=0 column halves as soon as y0 is
                                # written; only the j=1 halves trail v==3
                                nc.scalar.dma_start(out_d[s, i, 0:128, 0:TILES],
                                                    ym0[i][:, 0, :])
                                nc.scalar.dma_start(out_d[s, i, 128:192, 0:TILES],
                                                    ym1[i][:, 0, :])
                    elif v == 3:
                        last = s == S - 1
                        for i in range(2):
                            ks = list(enumerate(quads(i)))
                            if last and i == 1:
                                ks = ks[::-1]  # small M-chunk first
                            for k, (pm, ym, np_, bcol) in ks:
                                nc.vector.tensor_tensor(
                                    ym[:, 1, :], t1[(i, k)][:], pm[:, 3, :], sub)
                                if last and i == 1:
                                    dst = (out_d[s, i, 0:128, TILES:2 * TILES]
                                           if np_ == 128 else
                                           out_d[s, i, 128:192, TILES:2 * TILES])
                                    nc.scalar.dma_start(dst, ym[:, 1, :])
                            if not (last and i == 1):
                                nc.scalar.dma_start(out_d[s, i, 0:128, :], ym0[i][:])
                                nc.scalar.dma_start(out_d[s, i, 128:192, :], ym1[i][:])

            emit_input_dmas(0)
            emit_input_dmas(1)
            for s in range(S):
                emit_sample(s)

    nc.compile()
    return nc


def get_module():
    if "nc" not in _cache:
        _cache["nc"] = _build_module()
    return _cache["nc"]


def _route(x, gate_w, gate_b):
    """Replicates the reference router in numpy fp32. Returns combine [B,E]."""
    pooled = x.mean(axis=(2, 3), dtype=np.float32)
    logits = pooled @ gate_w + gate_b
    z = logits - logits.max(axis=-1, keepdims=True)
    ez = np.exp(z)
    w = ez / ez.sum(axis=-1, keepdims=True)
    topi = np.argsort(-w, axis=-1, kind="stable")[:, :TOPK]
    topw = np.take_along_axis(w, topi, axis=-1)
    topw = topw / (topw.sum(-1, keepdims=True) + 1e-10)
    combine = np.zeros((B, E), np.float32)
    np.put_along_axis(combine, topi, topw, axis=-1)
    return combine


def make_in_maps(x, gate_w, gate_b, expert_w, expert_b):
    import ml_dtypes
    bf16 = ml_dtypes.bfloat16

    x = np.ascontiguousarray(np.asarray(x, np.float32))
    gate_w = np.asarray(gate_w, np.float32)
    gate_b = np.asarray(gate_b, np.float32)
    expert_w = np.asarray(expert_w, np.float32)
    expert_b = np.asarray(expert_b, np.float32)

    combine = _route(x, gate_w, gate_b)                       # [B,E]
    Wc = np.einsum("be,eoikl->boikl", combine, expert_w)      # [B,C,C,3,3]
    bc = combine @ expert_b                                   # [B,C]

    # Weight transform U[b,u,v,c,o] (lhsT layout: K=in-ch, M=out-ch)
    U = np.einsum("uk,bockl,vl->buvco", _G, Wc, _G)

    # Input transform V[b,u,v,c,r,t]
    xp = np.zeros((B, C, H + 2, W + 2), np.float32)
    xp[:, :, 1:H + 1, 1:W + 1] = x
    d = np.empty((B, 4, 4, C, NT, NT), np.float32)
    for u in range(4):
        for v in range(4):
            d[:, u, v] = xp[:, :, u:u + 2 * NT:2, v:v + 2 * NT:2]
    V = np.einsum("uk,bklcrt,vl->buvcrt", _Bt, d, _Bt)

    # V super-stack [b, v, row=(u*C+c), tile] -> [b, v, part, chunk, tile]
    vstk = V.transpose(0, 2, 1, 3, 4, 5).reshape(B, 4, 4 * C, TILES)
    vstk = vstk.reshape(B, 4, 6, 128, TILES).transpose(0, 1, 3, 2, 4)
    vstk = np.ascontiguousarray(vstk.astype(bf16))            # [B,4,128,6,T]

    # U stacks with A^T row-fold signs baked in. ust[b, v, slot, part, o]
    # Slots: 0..3 = +U0,+U1,+U2 rows 0..511; slot4[0:64] = +U2 rows 512..575;
    # slots 5..7 = [-U2, -U3] rows re-aligned to V chunk partition offsets.
    Uall = U.transpose(0, 2, 1, 3, 4).reshape(B, 4, 4 * C, C)  # rows (u, c)
    ust = np.zeros((B, 4, 8, 128, C), np.float32)
    for k in range(4):
        ust[:, :, k] = Uall[:, :, 128 * k:128 * (k + 1)]
    ust[:, :, 4, 0:64] = Uall[:, :, 512:576]
    ust[:, :, 5] = -Uall[:, :, 384:512]
    ust[:, :, 6] = -Uall[:, :, 512:640]
    ust[:, :, 7] = -Uall[:, :, 640:768]
    ust = np.ascontiguousarray(ust.transpose(0, 1, 3, 2, 4).astype(bf16))
    # [B, 4, 128, 8, C]

    in_maps = []
    for c in range(NCORES):
        b0 = S * c
        bias = np.zeros((128, 2 * S), np.float32)
        for s in range(S):
            bias[:, 2 * s] = bc[b0 + s, 0:128]
            bias[0:64, 2 * s + 1] = bc[b0 + s, 128:192]
        in_maps.append({
            "vstk": np.ascontiguousarray(vstk[b0:b0 + S]),
            "ustk": np.ascontiguousarray(ust[b0:b0 + S]),
            "bias": bias,
        })
    return in_maps


def unshard_core(out_arr):
    """[S, 2, C, 2*TILES] bf16 -> [S, C, H, W] fp32 for one core."""
    a = np.asarray(out_arr, np.float32).reshape(S, 2, C, 2, NT, NT)
    full = np.empty((S, C, H, W), np.float32)
    for i in range(2):
        for j in range(2):
            full[:, :, i::2, j::2] = a[:, i, :, j]
    return full


def kernel(x, gate_w, gate_b, expert_w, expert_b):
    from concourse.bass_utils import run_bass_kernel_spmd

    nc = get_module()
    in_maps = make_in_maps(x, gate_w, gate_b, expert_w, expert_b)
    res = run_bass_kernel_spmd(nc, in_maps, core_ids=list(range(NCORES)))
    out = np.concatenate(
        [unshard_core(res.results[c]["out"]) for c in range(NCORES)])
    return out.reshape(B, C, H, W)
